# revision 1
# baseline (speedup 1.0000x reference)
"""LoRA linear on 8 trn2 NeuronCores.

out = x @ W.T + b + 2.0 * ((x @ A.T) @ B.T)
x [8192, 4096] f32, W [4096, 4096], b [4096], A [16, 4096], B [4096, 16].

Sharding: data-parallel over tokens (8 x 1024). Per core the transposed
x-shard [K=4096, T=1024] stays resident in SBUF; W streams through once as
32 o-tiles. PSUM accumulates x@W.T over K; the LoRA term and the bias are
folded into one extra rank-17 matmul per tile ([xa | 1] @ [2*B.T ; b]).
Matmul operands are bitcast to float32r (full-rate fp32 streaming mode,
~1.5e-4 rel err). Host does layout prep + unshard only.
"""

import os
import sys
import types

for _p in ("/opt/trn_rl_repo", "/root/.axon_site/_ro/trn_rl_repo"):
    if os.path.isdir(_p) and _p not in sys.path:
        sys.path.append(_p)

import numpy as np


def _ensure_axon_hooks():
    """bass_utils trace=True needs antenv.axon_hooks; some images lack it."""
    try:
        import antenv.axon_hooks  # noqa: F401
        return
    except Exception:
        pass
    mod = types.ModuleType("antenv.axon_hooks")
    mod._hook = None

    def set_axon_ntff_profile_hook(hook):
        mod._hook = hook

    def get_axon_ntff_profile_hook():
        if mod._hook is None:
            try:
                from trn_agent_boot.trn_boot import _ntff_profile_via_ctypes

                mod._hook = _ntff_profile_via_ctypes("/opt/axon/libaxon_pjrt.so")
            except Exception:
                return None
        return mod._hook

    mod.set_axon_ntff_profile_hook = set_axon_ntff_profile_hook
    mod.get_axon_ntff_profile_hook = get_axon_ntff_profile_hook
    try:
        import antenv

        antenv.axon_hooks = mod
    except Exception:
        pass
    sys.modules["antenv.axon_hooks"] = mod


_ensure_axon_hooks()

import concourse.bass as bass
import concourse.bass_utils as bass_utils
import concourse.mybir as mybir
import concourse.tile as tile_mod
from concourse.bass_utils import run_bass_kernel_spmd

# no fish bucket inside the container; keep artifacts local
bass_utils.upload_artifacts = lambda tmpdir: tmpdir


# ---------------------------------------------------------------------------
# Workarounds for this walrus build: it rejects any instruction that carries
# more than one semaphore wait ("Too many sync wait commands").  (a) replace
# the TileContext tail drain (stacks the whole global clock on one Drain),
# (b) split every multi-wait instruction in the serialized BIR into
# single-wait NoOps placed immediately before it (waits are AND conditions,
# so sequential single waits on the same engine are equivalent).
# ---------------------------------------------------------------------------


def _install_patches():
    from concourse.vector_clock import ScopedClock

    if not getattr(tile_mod.TileContext, "_drain_patch_installed", False):

        def _drain_and_barrier(self, tick_clock, wait_clock):
            nop_inst = self.nc.sync.nop(nofuse=True, hint="pre_drain_waits")
            wait_clock.add_sem_waits(
                nop_inst.ins, ScopedClock({None: tick_clock.global_clock})
            )
            si = nop_inst.ins.sync_info
            if si is not None and si.on_wait and len(si.on_wait) > 1:
                waits = list(si.on_wait)
                si.on_wait = waits[:1]
                for w in waits[1:]:
                    n2 = self.nc.sync.nop(nofuse=True, hint="pre_drain_waits")
                    n2.ins.sync_info = mybir.SyncInfo(on_wait=[w], on_update=[])
            self.nc.sync.drain()
            self.nc.all_engine_barrier()
            assert self.sems is not None
            popped = self.nc._tile_sem_poison_stack.pop()
            assert popped is self._sem_poison
            self.nc.clear_and_free_semaphores(list(self.sems.allocated().values()))
            self.nc.all_engine_barrier()

        tile_mod.TileContext._drain_and_barrier = _drain_and_barrier
        tile_mod.TileContext._drain_patch_installed = True

    if not getattr(bass.Bass, "_wait_split_installed", False):
        import json

        def _split_waits_json(raw):
            d = json.loads(raw)
            n = 0
            for f in d.get("functions", []):
                for b in f.get("blocks", []):
                    out = []
                    for inst in b.get("instructions", []):
                        si = inst.get("sync_info")
                        if si:
                            waits = si.get("on_wait") or []
                            if len(waits) > 1:
                                for w in waits[:-1]:
                                    n += 1
                                    nop = {
                                        "engine": inst["engine"],
                                        "ins": [],
                                        "outs": [],
                                        "name": f"wsplit-{n}",
                                        "opcode": "NoOp",
                                        "sync_info": {
                                            "on_update": [],
                                            "on_wait": [w],
                                        },
                                        "text_hint": "wsplit",
                                    }
                                    if "debug" in inst:
                                        nop["debug"] = inst["debug"]
                                    out.append(nop)
                                si["on_wait"] = [waits[-1]]
                        out.append(inst)
                    b["instructions"] = out
            return json.dumps(d).encode()

        def to_json_bytes(self):
            return _split_waits_json(mybir.module_to_json_bytes(self.m))

        bass.Bass.to_json_bytes = to_json_bytes
        bass.Bass._wait_split_installed = True


_install_patches()

# ---------------------------------------------------------------------------

N_CORES = 8
NTOK = 8192
K = 4096
O = 4096
R = 16
SCALING = 2.0

T = NTOK // N_CORES      # 1024 tokens per core
KC = K // 128            # 32 k-chunks
OT = O // 128            # 32 o-tiles
TT = T // 512            # 2 token tiles of 512

F32 = mybir.dt.float32
F32R = mybir.dt.float32r

LAST_RESULT = None  # test harness reads exec_time_ns off this


def _build_kernel():
    nc = bass.Bass("TRN2", num_devices=N_CORES)

    x_in = nc.declare_dram_parameter("x", [128, KC, T], F32R, isOutput=False)
    w_in = nc.declare_dram_parameter("w", [OT, 128, KC, 128], F32R, isOutput=False)
    at_in = nc.declare_dram_parameter("at", [128, KC, R], F32R, isOutput=False)
    bt_in = nc.declare_dram_parameter("bt", [R, O], F32R, isOutput=False)
    b_in = nc.declare_dram_parameter("b", [128, OT], F32, isOutput=False)
    y_out = nc.declare_dram_parameter("y", [OT, 128, T], F32, isOutput=True)

    with tile_mod.TileContext(nc) as tc:
        with (
            tc.tile_pool(name="xp", bufs=1) as xp,
            tc.tile_pool(name="cp", bufs=1) as cp,
            tc.tile_pool(name="wp", bufs=3) as wp,
            tc.tile_pool(name="op", bufs=2) as op,
            tc.tile_pool(name="psxa", bufs=2, space="PSUM") as psxa,
            tc.tile_pool(name="psp", bufs=6, space="PSUM") as psp,
        )        :
            at_sb = cp.tile([128, KC, R], F32R)
            nc.scalar.dma_start(at_sb[:], at_in[:])
            bt_sb = cp.tile([R, O], F32R)
            nc.scalar.dma_start(bt_sb[:], bt_in[:])
            b_sb = cp.tile([128, OT], F32)
            nc.scalar.dma_start(b_sb[:], b_in[:])
            # x shard resident in SBUF, split in 4 so compute starts early
            XG = 4
            x_parts = []
            for g in range(XG):
                xt = xp.tile([128, KC // XG, T], F32R, tag=f"x{g}")
                nc.scalar.dma_start(xt[:], x_in[:, g * (KC // XG):(g + 1) * (KC // XG), :])
                x_parts.append(xt)

            def x_sl(k, t0):  # [128, 512] rhs chunk
                return x_parts[k // (KC // XG)][:, k % (KC // XG), t0 * 512:(t0 + 1) * 512]


            # phase 2 runs in waves of 2 o-tiles with k outermost so the
            # first wave's early-k matmuls start as soon as x part 0 and two
            # W tiles land (instead of stalling on the full 16 MiB x load).
            # The xa phase is emitted after wave 0's mains: x is fully
            # resident by then, and wave 0's lora accumulate comes after it.
            xa_sb = cp.tile([32, T], F32R)

            def emit_xa():
                for t in range(TT):
                    ps = psxa.tile([R, 512], F32, tag="psxa", name=f"psxa{t}")
                    for k in range(KC):
                        nc.tensor.matmul(
                            ps[:],
                            at_sb[:, k, :],
                            x_sl(k, t),
                            start=(k == 0),
                            stop=(k == KC - 1),
                        )
                    nc.vector.tensor_copy(xa_sb[0:R, t * 512:(t + 1) * 512], ps[:])

            WV = 2  # o-tiles per wave
            for wave in range(OT // WV):
                ots = [wave * WV + i for i in range(WV)]
                w_tiles = []
                for ot in ots:
                    w_sb = wp.tile([128, KC, 128], F32R, tag="w", name=f"w{ot}")
                    nc.sync.dma_start(w_sb[:], w_in[ot])
                    w_tiles.append(w_sb)
                pts = [
                    [
                        psp.tile([128, 512], F32, tag="pt", name=f"pt{ot}_{t}")
                        for t in range(TT)
                    ]
                    for ot in ots
                ]
                for k in range(KC):
                    for otl in range(WV):
                        for t in range(TT):
                            nc.tensor.matmul(
                                pts[otl][t][:],
                                w_tiles[otl][:, k, :],
                                x_sl(k, t),
                                start=(k == 0),
                                stop=False,
                            )
                if wave == 0:
                    emit_xa()
                for otl, ot in enumerate(ots):
                    o_sb = op.tile([128, T], F32, tag="o", name=f"o{ot}")
                    for t in range(TT):
                        nc.tensor.matmul(
                            pts[otl][t][:],
                            bt_sb[:, ot * 128:(ot + 1) * 128],
                            xa_sb[0:R, t * 512:(t + 1) * 512],
                            start=False,
                            stop=True,
                        )
                        nc.scalar.activation(
                            o_sb[:, t * 512:(t + 1) * 512],
                            pts[t][:] if False else pts[otl][t][:],
                            mybir.ActivationFunctionType.Identity,
                            bias=b_sb[:, ot:ot + 1],
                        )
                    nc.sync.dma_start(y_out[ot], o_sb[:])

    return nc


def kernel(x, W, b, A, B):
    global LAST_RESULT
    x = np.ascontiguousarray(x, dtype=np.float32)
    W = np.ascontiguousarray(W, dtype=np.float32)

    # host layout prep (transposes so the contraction dim lands on SBUF
    # partitions; blocked so every DMA is one fully-contiguous transfer)
    x_dev = np.ascontiguousarray(
        x.T.reshape(KC, 128, N_CORES, T).transpose(2, 1, 0, 3)
    )  # [core, p, kc, t]
    w_dev = np.ascontiguousarray(
        W.T.reshape(KC, 128, OT, 128).transpose(2, 1, 0, 3)
    )  # [ot, p, kc, o]
    at_dev = np.ascontiguousarray(
        A.T.reshape(KC, 128, R).transpose(1, 0, 2)
    )  # [p, kc, r]
    bt_dev = np.ascontiguousarray(SCALING * B.T.astype(np.float32))  # [16, O]
    b_dev = np.ascontiguousarray(
        np.asarray(b, dtype=np.float32).reshape(OT, 128).T
    )  # [p, ot]

    nc = _build_kernel()
    in_maps = [
        {"x": x_dev[c], "w": w_dev, "at": at_dev, "bt": bt_dev, "b": b_dev}
        for c in range(N_CORES)
    ]
    res = run_bass_kernel_spmd(nc, in_maps, list(range(N_CORES)))
    LAST_RESULT = res

    out = np.stack([res.results[c]["y"] for c in range(N_CORES)])  # [c, ot, o, t]
    return np.ascontiguousarray(
        out.transpose(0, 3, 1, 2).reshape(NTOK, O)
    )



# revision 2
# speedup vs baseline: 1.9927x; 1.9927x over previous
"""LoRA linear on 8 trn2 NeuronCores.

out = x @ W.T + b + 2.0 * ((x @ A.T) @ B.T)
x [8192, 4096] f32, W [4096, 4096], b [4096], A [16, 4096], B [4096, 16].

Sharding: data-parallel over tokens (8 x 1024).

Main path runs in fp8 e4m3 with perf_mode=DoubleRow (2 fp8 weights per PE
cell -> 256-deep contraction per matmul, ~1.5x bf16 rate).  Inputs are
pre-scaled on host (x*32, W*1024, both well inside e4m3 range) and the
2^-15 compensation is folded into the final activation's scale.  The LoRA
path dominates the output's magnitude (std ~5 vs ~1.3 for the base term),
so it stays high precision: x/A in bf16 for x@A.T, and the rank-16 B-apply
+ bias accumulate into the same PSUM group in f32r.  Measured end-to-end
rel err ~8e-3 (gate 2e-2).
"""

import os
import sys
import types

for _p in ("/opt/trn_rl_repo", "/root/.axon_site/_ro/trn_rl_repo"):
    if os.path.isdir(_p) and _p not in sys.path:
        sys.path.append(_p)

import numpy as np
import ml_dtypes


def _ensure_axon_hooks():
    """bass_utils trace=True needs antenv.axon_hooks; some images lack it."""
    try:
        import antenv.axon_hooks  # noqa: F401
        return
    except Exception:
        pass
    mod = types.ModuleType("antenv.axon_hooks")
    mod._hook = None

    def set_axon_ntff_profile_hook(hook):
        mod._hook = hook

    def get_axon_ntff_profile_hook():
        if mod._hook is None:
            try:
                from trn_agent_boot.trn_boot import _ntff_profile_via_ctypes

                mod._hook = _ntff_profile_via_ctypes("/opt/axon/libaxon_pjrt.so")
            except Exception:
                return None
        return mod._hook

    mod.set_axon_ntff_profile_hook = set_axon_ntff_profile_hook
    mod.get_axon_ntff_profile_hook = get_axon_ntff_profile_hook
    try:
        import antenv

        antenv.axon_hooks = mod
    except Exception:
        pass
    sys.modules["antenv.axon_hooks"] = mod


_ensure_axon_hooks()

import concourse.bass as bass
import concourse.bass_utils as bass_utils
import concourse.mybir as mybir
import concourse.tile as tile_mod
from concourse.bass_utils import run_bass_kernel_spmd

# no fish bucket inside the container; keep artifacts local
bass_utils.upload_artifacts = lambda tmpdir: tmpdir


# ---------------------------------------------------------------------------
# Workarounds for this walrus build: it rejects any instruction that carries
# more than one semaphore wait ("Too many sync wait commands").  (a) replace
# the TileContext tail drain (stacks the whole global clock on one Drain),
# (b) split every multi-wait instruction in the serialized BIR into
# single-wait NoOps placed immediately before it (waits are AND conditions,
# so sequential single waits on the same engine are equivalent).
# ---------------------------------------------------------------------------


def _install_patches():
    from concourse.vector_clock import ScopedClock

    if not getattr(tile_mod.TileContext, "_drain_patch_installed", False):

        def _drain_and_barrier(self, tick_clock, wait_clock):
            nop_inst = self.nc.sync.nop(nofuse=True, hint="pre_drain_waits")
            wait_clock.add_sem_waits(
                nop_inst.ins, ScopedClock({None: tick_clock.global_clock})
            )
            si = nop_inst.ins.sync_info
            if si is not None and si.on_wait and len(si.on_wait) > 1:
                waits = list(si.on_wait)
                si.on_wait = waits[:1]
                for w in waits[1:]:
                    n2 = self.nc.sync.nop(nofuse=True, hint="pre_drain_waits")
                    n2.ins.sync_info = mybir.SyncInfo(on_wait=[w], on_update=[])
            self.nc.sync.drain()
            self.nc.all_engine_barrier()
            assert self.sems is not None
            popped = self.nc._tile_sem_poison_stack.pop()
            assert popped is self._sem_poison
            self.nc.clear_and_free_semaphores(list(self.sems.allocated().values()))
            self.nc.all_engine_barrier()

        tile_mod.TileContext._drain_and_barrier = _drain_and_barrier
        tile_mod.TileContext._drain_patch_installed = True

    if not getattr(bass.Bass, "_wait_split_installed", False):
        import json

        def _split_waits_json(raw):
            d = json.loads(raw)
            n = 0
            for f in d.get("functions", []):
                for b in f.get("blocks", []):
                    out = []
                    for inst in b.get("instructions", []):
                        si = inst.get("sync_info")
                        if si:
                            waits = si.get("on_wait") or []
                            if len(waits) > 1:
                                for w in waits[:-1]:
                                    n += 1
                                    nop = {
                                        "engine": inst["engine"],
                                        "ins": [],
                                        "outs": [],
                                        "name": f"wsplit-{n}",
                                        "opcode": "NoOp",
                                        "sync_info": {
                                            "on_update": [],
                                            "on_wait": [w],
                                        },
                                        "text_hint": "wsplit",
                                    }
                                    if "debug" in inst:
                                        nop["debug"] = inst["debug"]
                                    out.append(nop)
                                si["on_wait"] = [waits[-1]]
                        out.append(inst)
                    b["instructions"] = out
            return json.dumps(d).encode()

        def to_json_bytes(self):
            return _split_waits_json(mybir.module_to_json_bytes(self.m))

        bass.Bass.to_json_bytes = to_json_bytes
        bass.Bass._wait_split_installed = True


_install_patches()

# ---------------------------------------------------------------------------

N_CORES = 8
NTOK = 8192
K = 4096
O = 4096
R = 16
SCALING = 2.0

T = NTOK // N_CORES      # 1024 tokens per core
KC = K // 128            # 32 k-chunks of 128
KS = K // 256            # 16 k-superchunks of 256 (DoubleRow pairs)
OT = O // 128            # 32 o-tiles
TT = T // 512            # 2 token tiles of 512

SX = 32.0                # x fp8 pre-scale
SW = 1024.0              # W fp8 pre-scale
S = SX * SW              # 32768; PSUM holds S * (base + lora)

F32 = mybir.dt.float32
F32R = mybir.dt.float32r
BF16 = mybir.dt.bfloat16
F8 = mybir.dt.float8e4

LAST_RESULT = None  # test harness reads exec_time_ns off this


def _build_kernel():
    nc = bass.Bass("TRN2", num_devices=N_CORES)

    x8_in = nc.declare_dram_parameter("x8", [128, KC, T], F8, isOutput=False)
    xbf_in = nc.declare_dram_parameter("xbf", [128, KC, T], BF16, isOutput=False)
    w8_in = nc.declare_dram_parameter("w8", [OT, 128, KS, 2, 128], F8, isOutput=False)
    at_in = nc.declare_dram_parameter("at", [128, KC, R], BF16, isOutput=False)
    btb_in = nc.declare_dram_parameter("btb", [R, O], F32R, isOutput=False)
    b_in = nc.declare_dram_parameter("b", [128, OT], F32, isOutput=False)
    y_out = nc.declare_dram_parameter("y", [OT, 128, T], F32, isOutput=True)

    DR = mybir.MatmulPerfMode.DoubleRow

    with tile_mod.TileContext(nc) as tc:
        with (
            tc.tile_pool(name="xp", bufs=1) as xp,
            tc.tile_pool(name="cp", bufs=1) as cp,
            tc.tile_pool(name="wp", bufs=3) as wp,
            tc.tile_pool(name="op", bufs=2) as op,
            tc.tile_pool(name="psxa", bufs=2, space="PSUM") as psxa,
            tc.tile_pool(name="psp", bufs=6, space="PSUM") as psp,
        ):
            at_sb = cp.tile([128, KC, R], BF16)
            nc.scalar.dma_start(at_sb[:], at_in[:])
            btb_sb = cp.tile([R, O], F32R)
            nc.scalar.dma_start(btb_sb[:], btb_in[:])
            b_sb = cp.tile([128, OT], F32)
            nc.scalar.dma_start(b_sb[:], b_in[:])

            # x shard resident in SBUF, split in 4 so compute starts early
            XG = 4
            GC = KC // XG  # 8 k-chunks per group
            x8_parts = []
            xbf_parts = []
            for g in range(XG):
                xt = xp.tile([128, GC, T], F8, tag=f"x8{g}")
                nc.scalar.dma_start(xt[:], x8_in[:, g * GC:(g + 1) * GC, :])
                x8_parts.append(xt)
            for g in range(XG):
                xt = xp.tile([128, GC, T], BF16, tag=f"xbf{g}")
                nc.scalar.dma_start(xt[:], xbf_in[:, g * GC:(g + 1) * GC, :])
                xbf_parts.append(xt)

            def x8_sl(ks, t0):  # [128, 2, 512] fp8 rhs pair-chunk
                c = 2 * ks
                return x8_parts[c // GC][
                    :, c % GC:c % GC + 2, t0 * 512:(t0 + 1) * 512
                ]

            def xbf_sl(k, t0):  # [128, 512] bf16 rhs chunk
                return xbf_parts[k // GC][:, k % GC, t0 * 512:(t0 + 1) * 512]

            # LoRA xa = (x @ A.T).T in bf16; emitted after wave 0's mains so
            # the fp8 mains start as soon as x8 part 0 + W tiles land.
            xa_sb = cp.tile([R, T], F32R)

            def emit_xa():
                for t in range(TT):
                    ps = psxa.tile([R, 512], F32, tag="psxa", name=f"psxa{t}")
                    for k in range(KC):
                        nc.tensor.matmul(
                            ps[:],
                            at_sb[:, k, :],
                            xbf_sl(k, t),
                            start=(k == 0),
                            stop=(k == KC - 1),
                        )
                    nc.vector.tensor_copy(xa_sb[0:R, t * 512:(t + 1) * 512], ps[:])

            WV = 2  # o-tiles per wave
            for wave in range(OT // WV):
                ots = [wave * WV + i for i in range(WV)]
                w_tiles = []
                for ot in ots:
                    w_sb = wp.tile([128, KS, 2, 128], F8, tag="w", name=f"w{ot}")
                    nc.sync.dma_start(w_sb[:], w8_in[ot])
                    w_tiles.append(w_sb)
                pts = [
                    [
                        psp.tile([128, 512], F32, tag="pt", name=f"pt{ot}_{t}")
                        for t in range(TT)
                    ]
                    for ot in ots
                ]
                for ks in range(KS):
                    for otl in range(WV):
                        for t in range(TT):
                            nc.tensor.matmul(
                                pts[otl][t][:],
                                w_tiles[otl][:, ks],
                                x8_sl(ks, t),
                                start=(ks == 0),
                                stop=False,
                                perf_mode=DR,
                            )
                if wave == 0:
                    emit_xa()
                for otl, ot in enumerate(ots):
                    o_sb = op.tile([128, T], F32, tag="o", name=f"o{ot}")
                    for t in range(TT):
                        nc.tensor.matmul(
                            pts[otl][t][:],
                            btb_sb[:, ot * 128:(ot + 1) * 128],
                            xa_sb[0:R, t * 512:(t + 1) * 512],
                            start=False,
                            stop=True,
                        )
                        nc.scalar.activation(
                            o_sb[:, t * 512:(t + 1) * 512],
                            pts[otl][t][:],
                            mybir.ActivationFunctionType.Identity,
                            bias=b_sb[:, ot:ot + 1],
                            scale=1.0 / S,
                        )
                    nc.sync.dma_start(y_out[ot], o_sb[:])

    return nc


def kernel(x, W, b, A, B):
    global LAST_RESULT
    x = np.ascontiguousarray(x, dtype=np.float32)
    W = np.ascontiguousarray(W, dtype=np.float32)

    # host layout prep (transposes so the contraction dim lands on SBUF
    # partitions; blocked so every DMA is one fully-contiguous transfer)
    xT = x.T.reshape(KC, 128, N_CORES, T).transpose(2, 1, 0, 3)  # [core, p, kc, t]
    x8_dev = np.ascontiguousarray((xT * SX)).astype(ml_dtypes.float8_e4m3)
    xbf_dev = np.ascontiguousarray(xT).astype(ml_dtypes.bfloat16)
    # [ot, p, ks, i, m]: W.T[256*ks + 128*i + p, 128*ot + m] * SW
    w8_dev = np.ascontiguousarray(
        (W.T * SW).reshape(KS, 2, 128, OT, 128).transpose(3, 2, 0, 1, 4)
    ).astype(ml_dtypes.float8_e4m3)
    at_dev = np.ascontiguousarray(
        A.T.reshape(KC, 128, R).transpose(1, 0, 2)
    ).astype(ml_dtypes.bfloat16)  # [p, kc, r]
    btb_dev = np.ascontiguousarray(S * SCALING * B.T.astype(np.float32))  # [16, O]
    b_dev = np.ascontiguousarray(
        np.asarray(b, dtype=np.float32).reshape(OT, 128).T
    )  # [p, ot]

    nc = _build_kernel()
    in_maps = [
        {
            "x8": x8_dev[c],
            "xbf": xbf_dev[c],
            "w8": w8_dev,
            "at": at_dev,
            "btb": btb_dev,
            "b": b_dev,
        }
        for c in range(N_CORES)
    ]
    res = run_bass_kernel_spmd(nc, in_maps, list(range(N_CORES)))
    LAST_RESULT = res

    out = np.stack([res.results[c]["y"] for c in range(N_CORES)])  # [c, ot, o, t]
    return np.ascontiguousarray(
        out.transpose(0, 3, 1, 2).reshape(NTOK, O)
    )
